# revision 16
# baseline (speedup 1.0000x reference)
"""Block-circulant linear layer on TRN2 via one-level circulant CRT split.

y[n, j*B+k] = sum_{i,b} c[j,i,(k-b) mod B] * x[n, i*B+b] + bias[j*B+k]

Using x^256-1 = (x^128-1)(x^128+1): with u_i = x_i[:128]+x_i[128:],
v_i = x_i[:128]-x_i[128:], the op becomes two half-size dense systems
  yu = u @ U/2 + beta_u/2   (U: cyclic-128 block matrix, 2048x2048)
  yv = v @ V/2 + beta_v/2   (V: negacyclic-128 block matrix)
  y_lo = yu + yv, y_hi = yu - yv
— half the matmul FLOPs of the dense 4096x4096 form.

Sharding: data-parallel over the 8192 tokens (1024/core); U,V replicated.
fp32r (e8m11) matmul datapath at full PE rate; bias folded in via a K=1
ones-row matmul.
"""

import numpy as np

import concourse.bass as bass
import concourse.mybir as mybir
import concourse.tile as tile
from concourse import bacc
from concourse.bass_utils import run_bass_kernel_spmd

B = 256
H = B // 2               # 128
IN_BLOCKS = 16
OUT_BLOCKS = 16
BATCH, SEQ = 4, 2048
IN_F = IN_BLOCKS * B     # 4096
OUT_F = OUT_BLOCKS * B   # 4096
HF = IN_BLOCKS * H       # 2048 (half-system width)
N_CORES = 8
NTOK = BATCH * SEQ       # 8192
TOK = NTOK // N_CORES    # 1024 tokens per core

KT = HF // 128           # 16 contraction tiles per system
MT = TOK // 128          # 8 token tiles
NW = 512                 # moving free dim per matmul (one psum bank)
NT = HF // NW            # 4 column chunks per system
JB = NW // H             # 4 j-blocks per column chunk

_NC_CACHE = {}


def _build_nc():
    f32 = mybir.dt.float32
    f32r = mybir.dt.float32r

    nc = bacc.Bacc("TRN2", target_bir_lowering=False, debug=False)
    uT = nc.dram_tensor("uT", [HF, TOK], f32r, kind="ExternalInput")
    vT = nc.dram_tensor("vT", [HF, TOK], f32r, kind="ExternalInput")
    wU = nc.dram_tensor("wU", [NT, KT, 128, NW], f32r, kind="ExternalInput")
    wV = nc.dram_tensor("wV", [NT, KT, 128, NW], f32r, kind="ExternalInput")
    betaU = nc.dram_tensor("betaU", [1, HF], f32r, kind="ExternalInput")
    betaV = nc.dram_tensor("betaV", [1, HF], f32r, kind="ExternalInput")
    ones = nc.dram_tensor("ones", [1, TOK], f32r, kind="ExternalInput")
    # y stored as raw tiles (n, m, lo/hi, 128, NW); host reassembles
    y = nc.dram_tensor(
        "y", [NT, MT, 2, 128, NW], f32, kind="ExternalOutput"
    )

    with tile.TileContext(nc) as tc:
        with (
            tc.tile_pool(name="uvpool", bufs=1) as uvpool,
            tc.tile_pool(name="cpool", bufs=1) as cpool,
            tc.tile_pool(name="wpool", bufs=6) as wpool,
            tc.tile_pool(name="epool", bufs=8) as epool,
            tc.tile_pool(name="ypool", bufs=2) as ypool,
            tc.tile_pool(name="psum", bufs=8, space="PSUM") as psum_pool,
        ):
            # Resident u/v k-tiles (host-computed butterfly, feat-major).
            # u first: the U-phase consumes them immediately at kernel start.
            us, vs = [], []
            for i in range(IN_BLOCKS):
                ut = uvpool.tile([128, TOK], f32r, tag=f"u{i}", name=f"u{i}")
                nc.sync.dma_start(out=ut[:], in_=uT[i * 128 : (i + 1) * 128, :])
                us.append(ut)
            for i in range(IN_BLOCKS):
                vt = uvpool.tile([128, TOK], f32r, tag=f"v{i}", name=f"v{i}")
                nc.sync.dma_start(out=vt[:], in_=vT[i * 128 : (i + 1) * 128, :])
                vs.append(vt)

            ones_sb = cpool.tile([1, TOK], f32r, tag="ones")
            nc.sync.dma_start(out=ones_sb[:], in_=ones[:, :])
            betaU_sb = cpool.tile([1, HF], f32r, tag="bU")
            nc.sync.dma_start(out=betaU_sb[:], in_=betaU[:, :])
            betaV_sb = cpool.tile([1, HF], f32r, tag="bV")
            nc.sync.dma_start(out=betaV_sb[:], in_=betaV[:, :])

            for n in range(NT):
                nsl = slice(n * NW, (n + 1) * NW)
                # --- U phase ---
                psU = [
                    psum_pool.tile([128, NW], f32, tag="ps", name=f"pu_{n}_{m}")
                    for m in range(MT)
                ]
                for k in range(KT):
                    wt = wpool.tile([128, NW], f32r, tag="w", name=f"wu_{n}_{k}")
                    nc.gpsimd.dma_start(out=wt[:], in_=wU[n, k, :, :])
                    for m in range(MT):
                        nc.tensor.matmul(
                            psU[m][:],
                            us[k][:, m * 128 : (m + 1) * 128],
                            wt[:],
                            start=(k == 0),
                            stop=False,
                        )
                for m in range(MT):
                    nc.tensor.matmul(
                        psU[m][:],
                        ones_sb[:, m * 128 : (m + 1) * 128],
                        betaU_sb[:, nsl],
                        start=False,
                        stop=True,
                    )
                yus = []
                for m in range(MT):
                    yu = epool.tile([128, NW], f32, tag="yu", name=f"yu_{n}_{m}")
                    nc.vector.tensor_copy(yu[:], psU[m][:])
                    yus.append(yu)
                # --- V phase ---
                psV = [
                    psum_pool.tile([128, NW], f32, tag="ps", name=f"pv_{n}_{m}")
                    for m in range(MT)
                ]
                for k in range(KT):
                    wt = wpool.tile([128, NW], f32r, tag="w", name=f"wv_{n}_{k}")
                    nc.gpsimd.dma_start(out=wt[:], in_=wV[n, k, :, :])
                    for m in range(MT):
                        nc.tensor.matmul(
                            psV[m][:],
                            vs[k][:, m * 128 : (m + 1) * 128],
                            wt[:],
                            start=(k == 0),
                            stop=False,
                        )
                for m in range(MT):
                    nc.tensor.matmul(
                        psV[m][:],
                        ones_sb[:, m * 128 : (m + 1) * 128],
                        betaV_sb[:, nsl],
                        start=False,
                        stop=True,
                    )
                # --- recombine + store ---
                for m in range(MT):
                    tlo = ypool.tile([128, NW], f32, tag="tlo", name=f"tlo_{n}_{m}")
                    thi = ypool.tile([128, NW], f32, tag="thi", name=f"thi_{n}_{m}")
                    nc.vector.tensor_add(tlo[:], yus[m][:], psV[m][:])
                    nc.vector.tensor_sub(thi[:], yus[m][:], psV[m][:])
                    nc.sync.dma_start(out=y[n, m, 0, :, :], in_=tlo[:])
                    nc.sync.dma_start(out=y[n, m, 1, :, :], in_=thi[:])
    nc.finalize()
    return nc


def _get_nc():
    if "nc" not in _NC_CACHE:
        _NC_CACHE["nc"] = _build_nc()
    return _NC_CACHE["nc"]


def _round_fp32r(a: np.ndarray) -> np.ndarray:
    """Round fp32 to fp32r (e8m11: low 12 mantissa bits zero), RNE."""
    u = np.ascontiguousarray(a, dtype=np.float32).view(np.uint32)
    r = (u + (0x7FF + ((u >> 12) & 1))) & np.uint32(0xFFFFF000)
    return r.view(np.float32)


def _build_weights(c: np.ndarray, bias: np.ndarray):
    # cyclic/negacyclic half-size blocks
    cu = c[:, :, :H] + c[:, :, H:]                         # (J, I, H)
    cv = c[:, :, :H] - c[:, :, H:]
    kk = np.arange(H)
    bb = np.arange(H)
    idx = (kk[None, :] - bb[:, None]) % H                  # (bb, kk)
    sign = np.where(kk[None, :] >= bb[:, None], 1.0, -1.0).astype(np.float32)
    U = cu[:, :, idx].transpose(1, 2, 0, 3).reshape(HF, HF) * 0.5
    V = (cv[:, :, idx] * sign[None, None]).transpose(1, 2, 0, 3).reshape(
        HF, HF
    ) * 0.5
    bias_b = bias.reshape(OUT_BLOCKS, B)
    beta_u = 0.5 * (bias_b[:, :H] + bias_b[:, H:]).reshape(1, HF)
    beta_v = 0.5 * (bias_b[:, :H] - bias_b[:, H:]).reshape(1, HF)
    def tiled(w):
        # (HF, HF) -> (NT, KT, 128, NW) so each [128, NW] tile is contiguous
        return np.ascontiguousarray(
            w.reshape(KT, 128, NT, NW).transpose(2, 0, 1, 3)
        )

    return (
        _round_fp32r(tiled(U)),
        _round_fp32r(tiled(V)),
        _round_fp32r(beta_u),
        _round_fp32r(beta_v),
    )


def kernel(x, c, bias, _spmd_kwargs=None):
    x = np.asarray(x, dtype=np.float32)
    c = np.asarray(c, dtype=np.float32)
    bias = np.asarray(bias, dtype=np.float32)

    wu, wv, bu, bv = _build_weights(c, bias)
    ones = np.ones((1, TOK), dtype=np.float32)
    xb = x.reshape(NTOK, IN_BLOCKS, B)
    u_all = (xb[:, :, :H] + xb[:, :, H:]).reshape(NTOK, HF)
    v_all = (xb[:, :, :H] - xb[:, :, H:]).reshape(NTOK, HF)

    in_maps = []
    for cid in range(N_CORES):
        sl = slice(cid * TOK, (cid + 1) * TOK)
        in_maps.append(
            {
                "uT": _round_fp32r(u_all[sl].T),           # (HF, TOK)
                "vT": _round_fp32r(v_all[sl].T),
                "wU": wu,
                "wV": wv,
                "betaU": bu,
                "betaV": bv,
                "ones": ones,
            }
        )

    nc = _get_nc()
    kw = dict(_spmd_kwargs or {})
    one_core = kw.pop("_one_core", False)
    if one_core:
        res = run_bass_kernel_spmd(nc, in_maps[:1], core_ids=[0], **kw)
        return None, res
    res = run_bass_kernel_spmd(
        nc, in_maps, core_ids=list(range(N_CORES)), **kw
    )
    def reassemble(a):
        # (NT, MT, 2, 128, NW) -> (TOK, OUT_F)
        a = a.reshape(NT, MT, 2, 128, JB, H)
        return a.transpose(1, 3, 0, 4, 2, 5).reshape(TOK, OUT_F)

    y = np.concatenate([reassemble(r["y"]) for r in res.results], axis=0)
    out = y.reshape(BATCH, SEQ, OUT_F)
    if _spmd_kwargs:
        return out, res
    return out


# revision 19
# speedup vs baseline: 1.0704x; 1.0704x over previous
"""Block-circulant linear layer on TRN2 via two-level circulant CRT split.

y[n, j*B+k] = sum_{i,b} c[j,i,(k-b) mod B] * x[n, i*B+b] + bias[j*B+k]

Level 1: x^256-1 = (x^128-1)(x^128+1) -> cyclic-128 system U (on u) and
negacyclic-128 system V (on v). Level 2 splits U again:
x^128-1 = (x^64-1)(x^64+1) -> UU (cyclic-64, on uu), UV (negacyclic-64,
on uv). Matmul FLOPs drop to 3/8 of the dense 4096x4096 form:
  yv  = v  @ V/2  + beta_v    (2048x2048)
  yuu = uu @ UU/4 + beta_uu   (1024x1024)
  yuv = uv @ UV/4 + beta_uv   (1024x1024)
  yu_lo = yuu + yuv, yu_hi = yuu - yuv          (stage A)
  y_lo = yu + yv, y_hi = yu - yv                (stage B)

Sharding: data-parallel over the 8192 tokens (1024/core); weights
replicated. fp32r (e8m11) matmul datapath; bias folded in via K=1
ones-row matmuls; input butterflies/transpose and output reassembly are
host-side data marshalling.
"""

import numpy as np

import concourse.bass as bass
import concourse.mybir as mybir
import concourse.tile as tile
from concourse import bacc
from concourse.bass_utils import run_bass_kernel_spmd

B = 256
H = B // 2               # 128
Q = B // 4               # 64
IN_BLOCKS = 16
OUT_BLOCKS = 16
BATCH, SEQ = 4, 2048
IN_F = IN_BLOCKS * B     # 4096
OUT_F = OUT_BLOCKS * B   # 4096
HF = IN_BLOCKS * H       # 2048 (V system width)
QF = IN_BLOCKS * Q       # 1024 (UU/UV system width)
N_CORES = 8
NTOK = BATCH * SEQ       # 8192
TOK = NTOK // N_CORES    # 1024 tokens per core

KTV = HF // 128          # 16 contraction tiles, V system
KTQ = QF // 128          # 8 contraction tiles, UU/UV systems
MT = TOK // 128          # 8 token tiles
NW = 512                 # moving free dim per matmul (one psum bank)
NTV = HF // NW           # 4 column chunks, V system
NTQ = QF // NW           # 2 column chunks, UU/UV systems
JB = NW // H             # 4 j-blocks per V/output chunk

_NC_CACHE = {}


def _build_nc():
    f32 = mybir.dt.float32
    f32r = mybir.dt.float32r

    nc = bacc.Bacc("TRN2", target_bir_lowering=False, debug=False)
    vT = nc.dram_tensor("vT", [HF, TOK], f32r, kind="ExternalInput")
    uuT = nc.dram_tensor("uuT", [QF, TOK], f32r, kind="ExternalInput")
    uvT = nc.dram_tensor("uvT", [QF, TOK], f32r, kind="ExternalInput")
    wV = nc.dram_tensor("wV", [NTV, KTV, 128, NW], f32r, kind="ExternalInput")
    wUU = nc.dram_tensor("wUU", [NTQ, KTQ, 128, NW], f32r, kind="ExternalInput")
    wUV = nc.dram_tensor("wUV", [NTQ, KTQ, 128, NW], f32r, kind="ExternalInput")
    # rows: 0 = beta_v (2048), 32 = beta_uu (1024 + pad), 64 = beta_uv
    # (matmul operands must start at partition 0, 32, or 64)
    betaAll = nc.dram_tensor("betaAll", [65, HF], f32r, kind="ExternalInput")
    ones = nc.dram_tensor("ones", [65, 128], f32r, kind="ExternalInput")
    # y stored as raw stage-B tiles (n, m, lo/hi, 128, NW); host reassembles
    y = nc.dram_tensor(
        "y", [NTV, MT, 2, 128, NW], f32, kind="ExternalOutput"
    )

    with tile.TileContext(nc) as tc:
        with (
            tc.tile_pool(name="inpool", bufs=1) as inpool,
            tc.tile_pool(name="cpool", bufs=1) as cpool,
            tc.tile_pool(name="wpool", bufs=4) as wpool,
            tc.tile_pool(name="yupool", bufs=8) as yupool,
            tc.tile_pool(name="ypool", bufs=2) as ypool,
            tc.tile_pool(name="psum", bufs=8, space="PSUM") as psum_pool,
        ):
            # Resident input k-tiles (host-computed butterflies, feat-major).
            # uu/uv first: the UU phase consumes them at kernel start.
            uus, uvs, vs = [], [], []
            for i in range(KTQ):
                t = inpool.tile([128, TOK], f32r, tag=f"uu{i}", name=f"uu{i}")
                nc.sync.dma_start(out=t[:], in_=uuT[i * 128 : (i + 1) * 128, :])
                uus.append(t)
            for i in range(KTQ):
                t = inpool.tile([128, TOK], f32r, tag=f"uv{i}", name=f"uv{i}")
                nc.sync.dma_start(out=t[:], in_=uvT[i * 128 : (i + 1) * 128, :])
                uvs.append(t)
            for i in range(KTV):
                t = inpool.tile([128, TOK], f32r, tag=f"v{i}", name=f"v{i}")
                nc.sync.dma_start(out=t[:], in_=vT[i * 128 : (i + 1) * 128, :])
                vs.append(t)

            ones_sb = cpool.tile([65, 128], f32r, tag="ones")
            nc.sync.dma_start(out=ones_sb[:], in_=ones[:, :])
            beta_sb = cpool.tile([65, HF], f32r, tag="beta")
            nc.sync.dma_start(out=beta_sb[:], in_=betaAll[:, :])

            def system_phase(tag, ktiles, lhs_tiles, wdram, nn, beta_row):
                """One accumulation phase: psum[m] = sum_k lhsT_k.T @ W + beta."""
                ps = [
                    psum_pool.tile(
                        [128, NW], f32, tag="ps", name=f"ps_{tag}_{nn}_{m}"
                    )
                    for m in range(MT)
                ]
                for k in range(ktiles):
                    wt = wpool.tile(
                        [128, NW], f32r, tag="w", name=f"w_{tag}_{nn}_{k}"
                    )
                    nc.gpsimd.dma_start(out=wt[:], in_=wdram[nn, k, :, :])
                    for m in range(MT):
                        nc.tensor.matmul(
                            ps[m][:],
                            lhs_tiles[k][:, m * 128 : (m + 1) * 128],
                            wt[:],
                            start=(k == 0),
                            stop=False,
                        )
                for m in range(MT):
                    nc.tensor.matmul(
                        ps[m][:],
                        ones_sb[beta_row : beta_row + 1, :],
                        beta_sb[beta_row : beta_row + 1, nn * NW : (nn + 1) * NW],
                        start=False,
                        stop=True,
                    )
                return ps

            for nn in range(NTQ):
                psUU = system_phase("uu", KTQ, uus, wUU, nn, 32)
                ylo = []
                for m in range(MT):
                    t = yupool.tile([128, NW], f32, tag="ylo", name=f"ylo_{nn}_{m}")
                    nc.vector.tensor_copy(t[:], psUU[m][:])
                    ylo.append(t)
                psUV = system_phase("uv", KTQ, uvs, wUV, nn, 64)
                yhi = []
                for m in range(MT):
                    t = yupool.tile([128, NW], f32, tag="yhi", name=f"yhi_{nn}_{m}")
                    nc.vector.tensor_sub(t[:], ylo[m][:], psUV[m][:])
                    # in-place promote: ylo becomes yuu + yuv (stage A)
                    nc.vector.tensor_add(ylo[m][:], ylo[m][:], psUV[m][:])
                    yhi.append(t)
                for h in range(2):
                    n = 2 * nn + h
                    psV = system_phase("v", KTV, vs, wV, n, 0)
                    for m in range(MT):
                        tlo = ypool.tile(
                            [128, NW], f32, tag="tlo", name=f"tlo_{n}_{m}"
                        )
                        thi = ypool.tile(
                            [128, NW], f32, tag="thi", name=f"thi_{n}_{m}"
                        )
                        # 3D views: stage-A tiles are (j8, kk64); psV/y are (j4, kk128)
                        ylo3 = ylo[m][:].rearrange("p (j k) -> p j k", k=Q)
                        yhi3 = yhi[m][:].rearrange("p (j k) -> p j k", k=Q)
                        psv3 = psV[m][:].rearrange("p (j k) -> p j k", k=H)
                        tlo3 = tlo[:].rearrange("p (j k) -> p j k", k=H)
                        thi3 = thi[:].rearrange("p (j k) -> p j k", k=H)
                        slo = ylo3[:, 4 * h : 4 * h + 4, :]
                        shi = yhi3[:, 4 * h : 4 * h + 4, :]
                        nc.vector.tensor_add(
                            tlo3[:, :, 0:Q], slo, psv3[:, :, 0:Q]
                        )
                        nc.vector.tensor_add(
                            tlo3[:, :, Q:H], shi, psv3[:, :, Q:H]
                        )
                        nc.vector.tensor_sub(
                            thi3[:, :, 0:Q], slo, psv3[:, :, 0:Q]
                        )
                        nc.vector.tensor_sub(
                            thi3[:, :, Q:H], shi, psv3[:, :, Q:H]
                        )
                        nc.sync.dma_start(out=y[n, m, 0, :, :], in_=tlo[:])
                        nc.sync.dma_start(out=y[n, m, 1, :, :], in_=thi[:])
    nc.finalize()
    return nc


def _get_nc():
    if "nc" not in _NC_CACHE:
        _NC_CACHE["nc"] = _build_nc()
    return _NC_CACHE["nc"]


def _round_fp32r(a: np.ndarray) -> np.ndarray:
    """Round fp32 to fp32r (e8m11: low 12 mantissa bits zero), RNE."""
    u = np.ascontiguousarray(a, dtype=np.float32).view(np.uint32)
    r = (u + (0x7FF + ((u >> 12) & 1))) & np.uint32(0xFFFFF000)
    return r.view(np.float32)


def _cyc(cm, n):
    k = np.arange(n)
    b = np.arange(n)
    return cm[:, :, (k[None] - b[:, None]) % n]


def _neg(cm, n):
    k = np.arange(n)
    b = np.arange(n)
    s = np.where(k[None] >= b[:, None], 1.0, -1.0).astype(np.float32)
    return cm[:, :, (k[None] - b[:, None]) % n] * s[None, None]


def _flat(blk, n):
    # (j, i, bb, kk) -> (I*n, J*n)
    return blk.transpose(1, 2, 0, 3).reshape(IN_BLOCKS * n, OUT_BLOCKS * n)


def _tiled(w, nt, kt):
    # (K, N) -> (nt, kt, 128, NW): each [128, NW] tile contiguous
    return np.ascontiguousarray(
        w.reshape(kt, 128, nt, NW).transpose(2, 0, 1, 3)
    )


def _build_weights(c: np.ndarray, bias: np.ndarray):
    cu = c[:, :, :H] + c[:, :, H:]
    cv = c[:, :, :H] - c[:, :, H:]
    cuu = cu[:, :, :Q] + cu[:, :, Q:]
    cuv = cu[:, :, :Q] - cu[:, :, Q:]

    V = _flat(_neg(cv, H), H) * 0.5
    UU = _flat(_cyc(cuu, Q), Q) * 0.25
    UV = _flat(_neg(cuv, Q), Q) * 0.25

    bias_b = bias.reshape(OUT_BLOCKS, B)
    bu = 0.5 * (bias_b[:, :H] + bias_b[:, H:])           # (J, H)
    bv = 0.5 * (bias_b[:, :H] - bias_b[:, H:]).reshape(OUT_BLOCKS * H)
    buu = 0.5 * (bu[:, :Q] + bu[:, Q:]).reshape(OUT_BLOCKS * Q)
    buv = 0.5 * (bu[:, :Q] - bu[:, Q:]).reshape(OUT_BLOCKS * Q)
    betaAll = np.zeros((65, HF), np.float32)
    betaAll[0] = bv
    betaAll[32, :QF] = buu
    betaAll[64, :QF] = buv

    return (
        _round_fp32r(_tiled(V, NTV, KTV)),
        _round_fp32r(_tiled(UU, NTQ, KTQ)),
        _round_fp32r(_tiled(UV, NTQ, KTQ)),
        _round_fp32r(betaAll),
    )


def kernel(x, c, bias, _spmd_kwargs=None):
    x = np.asarray(x, dtype=np.float32)
    c = np.asarray(c, dtype=np.float32)
    bias = np.asarray(bias, dtype=np.float32)

    wv, wuu, wuv, betas = _build_weights(c, bias)
    ones = np.ones((65, 128), dtype=np.float32)

    xb = x.reshape(NTOK, IN_BLOCKS, B)
    u = xb[:, :, :H] + xb[:, :, H:]                      # (NTOK, I, H)
    v_all = (xb[:, :, :H] - xb[:, :, H:]).reshape(NTOK, HF)
    uu_all = (u[:, :, :Q] + u[:, :, Q:]).reshape(NTOK, QF)
    uv_all = (u[:, :, :Q] - u[:, :, Q:]).reshape(NTOK, QF)

    in_maps = []
    for cid in range(N_CORES):
        sl = slice(cid * TOK, (cid + 1) * TOK)
        in_maps.append(
            {
                "vT": _round_fp32r(v_all[sl].T),         # (HF, TOK)
                "uuT": _round_fp32r(uu_all[sl].T),       # (QF, TOK)
                "uvT": _round_fp32r(uv_all[sl].T),
                "wV": wv,
                "wUU": wuu,
                "wUV": wuv,
                "betaAll": betas,
                "ones": ones,
            }
        )

    nc = _get_nc()
    kw = dict(_spmd_kwargs or {})
    one_core = kw.pop("_one_core", False)
    if one_core:
        res = run_bass_kernel_spmd(nc, in_maps[:1], core_ids=[0], **kw)
        return None, res

    res = run_bass_kernel_spmd(
        nc, in_maps, core_ids=list(range(N_CORES)), **kw
    )

    def reassemble(a):
        # (NTV, MT, 2, 128, NW) -> (TOK, OUT_F)
        a = a.reshape(NTV, MT, 2, 128, JB, H)
        return a.transpose(1, 3, 0, 4, 2, 5).reshape(TOK, OUT_F)

    y = np.concatenate([reassemble(r["y"]) for r in res.results], axis=0)
    out = y.reshape(BATCH, SEQ, OUT_F)
    if _spmd_kwargs:
        return out, res
    return out
